# revision 30
# baseline (speedup 1.0000x reference)
"""Row-wise Pearson correlation kernel for Trainium2 (Bass/Tile).

Full inputs v1, v2: [262144, 256] f32. Output: [262144] f32 where
out[r] = (E[xy] - E[x]E[y]) / sqrt(var_s(x) * var_s(y))   (sample var, ddof=1)

Sharding: rows split evenly across 8 NeuronCores (no communication needed).
Per core: 32768 rows -> 256 blocks of 128 rows (rows on partitions).

Per-core dataflow (memory-bound, ~155us/core = ~420 GB/s/core, at the HBM
roofline; engine work is split so every engine stays under the DMA time):
  - DMA supertiles [128, SUPER=8, 256] of v1 and v2 into SBUF (HWDGE)
  - VectorE bn_stats per 128-row block -> per-row (mean, M2) of v1 and of v2
  - GPSIMD computes w = v1 + v2 (keeps VectorE free)
  - ScalarE Square-activation with fused accum -> per-row sum(w^2)
  - Sxy is recovered in the final combine via the polarization identity
      Sxy = (sum(w^2) - Sxx - Syy) / 2
  - final combine on [128, 256] per-row stat tiles -> r; one DMA out per core
    (out[p, i] = r(row i*128+p); the host transposes back)
"""

import numpy as np

N_FULL = 262144
D = 256
N_CORES = 8
N_PER_CORE = N_FULL // N_CORES  # 32768
P = 128
NBLK = N_PER_CORE // P          # 256 row-blocks per core
SUPER = 8                       # row-blocks per supertile (one DMA / bn_stats batch)
NSUP = NBLK // SUPER

_NC_CACHE = None
LAYOUT = "rowmajor"  # or "colmajor"
IN_DTYPE = "float16"  # HBM storage dtype for v1/v2 (host casts f32 inputs)


def to_input_dtype(arr):
    import numpy as np
    if IN_DTYPE == "float16":
        return np.ascontiguousarray(arr, dtype=np.float16)
    if IN_DTYPE == "bfloat16":
        import ml_dtypes
        return np.ascontiguousarray(np.asarray(arr).astype(ml_dtypes.bfloat16))
    return np.ascontiguousarray(arr, dtype=np.float32)


def _build_nc(passes=1, dma_only=False, compute_only=False,
              super_=None, data_bufs=6, act_k=1, scratch_bufs=4, dma_split=1,
              layout=None, act_psum=False, inplace_sq=False, gp_split=1,
              hw_loop=True, unroll_reps=1, dve_k=0, in_dtype=None,
              no_dve=False, no_act=False, no_gp=False, bn_batch=1,
              probe=None):
    """passes>1 with hw_loop=True wraps the streaming loop in a tc.For_i
    hardware loop (constant program size, used for slope timing)."""
    import contextlib
    from concourse import bacc, mybir
    import concourse.tile as tile

    f32 = mybir.dt.float32
    in_dtype = in_dtype if in_dtype is not None else IN_DTYPE
    fin = getattr(mybir.dt, in_dtype)
    SUPER = super_ if super_ is not None else globals()["SUPER"]
    NSUP = NBLK // SUPER
    nc = bacc.Bacc(None, target_bir_lowering=False, debug=False)

    v1 = nc.dram_tensor("v1", [N_PER_CORE, D], fin, kind="ExternalInput")
    v2 = nc.dram_tensor("v2", [N_PER_CORE, D], fin, kind="ExternalInput")
    # out[p, i] = r(row i*128 + p); host transposes back
    out = nc.dram_tensor("out", [P, NBLK], f32, kind="ExternalOutput")

    layout = layout if layout is not None else LAYOUT
    if layout == "colmajor":
        # out[p, i] = r(row i*128 + p); per-partition DRAM reads are 1KB chunks
        v1r = v1[:].rearrange("(n p) d -> p n d", p=P)  # [128, NBLK, D]
        v2r = v2[:].rearrange("(n p) d -> p n d", p=P)
    else:
        # rowmajor: out[p, i] = r(row p*NBLK + i); per-partition DRAM reads are
        # SUPER*1KB contiguous chunks (fewer, larger descriptor runs)
        v1r = v1[:].rearrange("(p n) d -> p n d", p=P)  # [128, NBLK, D]
        v2r = v2[:].rearrange("(p n) d -> p n d", p=P)

    with tile.TileContext(nc) as tc:
        with (
            tc.tile_pool(name="data", bufs=data_bufs) as data,
            tc.tile_pool(name="scratch", bufs=scratch_bufs) as scratch,
            tc.tile_pool(name="stats", bufs=1) as stats,
            tc.tile_pool(name="psum", bufs=2, space="PSUM") as psum,
        ):
            actpool = psum if act_psum else scratch
            s1 = stats.tile([P, NBLK, 6], f32)   # bn_stats(v1): per-block 6-tuple
            s2 = stats.tile([P, NBLK, 6], f32)
            sww = stats.tile([P, NBLK], f32)     # per-row sum((x+y)^2)
            syB = stats.tile([P, NBLK], f32)     # ACT-offloaded: raw sum(y)
            syyB = stats.tile([P, NBLK], f32)    # ACT-offloaded: raw sum(y^2)
            if dma_only or compute_only or no_dve or no_act:
                nc.vector.memset(s1, 1.0)
                nc.vector.memset(s2, 1.0)
                nc.vector.memset(sww, 1.0)
            if dma_only or compute_only or act_k > 0 or no_act:
                nc.vector.memset(syB, 1.0)
                nc.vector.memset(syyB, 1.0)

            if compute_only:
                t1c = data.tile([P, SUPER, D], fin, tag="t1")
                t2c = data.tile([P, SUPER, D], fin, tag="t2")
                nc.gpsimd.memset(t1c, 0.5)
                nc.gpsimd.memset(t2c, 0.25)

            if passes > 1 and hw_loop:
                rep_iter = range(unroll_reps)
                assert passes % unroll_reps == 0
                loop_cm = tc.For_i(0, passes // unroll_reps)
            else:
                rep_iter = range(passes)
                loop_cm = contextlib.nullcontext()
            with loop_cm:
             for _rep in rep_iter:
              for s in range(NSUP):
                blk = slice(s * SUPER, (s + 1) * SUPER)
                if compute_only:
                    t1, t2 = t1c, t2c
                else:
                    t1 = data.tile([P, SUPER, D], fin, tag="t1")
                    t2 = data.tile([P, SUPER, D], fin, tag="t2")
                    if dma_split <= 1:
                        nc.sync.dma_start(out=t1, in_=v1r[:, blk, :])
                        nc.sync.dma_start(out=t2, in_=v2r[:, blk, :])
                    else:
                        step = SUPER // dma_split
                        for j in range(dma_split):
                            jb = slice(s * SUPER + j * step, s * SUPER + (j + 1) * step)
                            jt = slice(j * step, (j + 1) * step)
                            nc.sync.dma_start(out=t1[:, jt, :], in_=v1r[:, jb, :])
                            nc.sync.dma_start(out=t2[:, jt, :], in_=v2r[:, jb, :])
                if dma_only:
                    continue

                if probe is not None:
                    # engine-rate microbenchmarks, one primitive per probe
                    if probe == "dve_reduce":
                        redout = scratch.tile([P, 3, SUPER], f32, tag="red")
                        for k, tt in enumerate((t1, t2, t1)):
                            nc.vector.tensor_reduce(
                                out=redout[:, k, :], in_=tt,
                                axis=mybir.AxisListType.X,
                                op=mybir.AluOpType.add)
                    elif probe == "dve_tt":
                        wp = scratch.tile([P, SUPER, D], fin, tag="w")
                        nc.vector.tensor_tensor(
                            out=wp, in0=t1, in1=t2, op=mybir.AluOpType.add)
                    elif probe == "act_batch":
                        wp = scratch.tile([P, SUPER, D], fin, tag="w")
                        wp2 = scratch.tile([P, SUPER, D], fin, tag="wsq")
                        nc.scalar.activation(
                            out=wp, in_=t1,
                            func=mybir.ActivationFunctionType.Square)
                        nc.scalar.activation(
                            out=wp2, in_=t2,
                            func=mybir.ActivationFunctionType.Square)
                    elif probe == "gp_mult":
                        wp = scratch.tile([P, SUPER, D], fin, tag="w")
                        nc.gpsimd.tensor_tensor(
                            out=wp, in0=t1, in1=t2, op=mybir.AluOpType.mult)
                    elif probe == "gp_stt":
                        wp = scratch.tile([P, SUPER, D], fin, tag="w")
                        for h in range(SUPER):
                            g = s * SUPER + h
                            nc.gpsimd.scalar_tensor_tensor(
                                out=wp[:, h, :], in0=t1[:, h, :], scalar=1.0,
                                in1=t2[:, h, :],
                                op0=mybir.AluOpType.mult,
                                op1=mybir.AluOpType.mult,
                                accum_out=sww[:, g : g + 1])
                    elif probe == "dve_bn2":
                        # one bn_stats over an interleaved 2-block view:
                        # even positions = block h, odd = block h+1.
                        # Bypasses the wrapper's segmented-shape check (the HW
                        # emits ONE 6-tuple per instruction over the stream).
                        for h in range(0, SUPER, 2):
                            g = s * SUPER + h
                            for tt, ss in ((t1, s1), (t2, s2)):
                                iv = tt[:, h : h + 2, :].rearrange("p h d -> p d h")
                                eng = nc.vector
                                eng.add_instruction(mybir.InstBNStats(
                                    name=nc.get_next_instruction_name(),
                                    ins=[eng.lower_ap(iv)],
                                    outs=[eng.lower_ap(ss[:, g, : 6])],
                                ))
                    elif probe == "dve_ttred":
                        wp = scratch.tile([P, SUPER, D], fin, tag="w")
                        for h in range(SUPER):
                            g = s * SUPER + h
                            nc.vector.tensor_tensor_reduce(
                                out=wp[:, h, :], in0=t1[:, h, :],
                                in1=t2[:, h, :], scale=1.0, scalar=0.0,
                                op0=mybir.AluOpType.mult,
                                op1=mybir.AluOpType.add,
                                accum_out=sww[:, g : g + 1])
                    else:
                        raise ValueError(probe)
                    continue

                # bn_stats output must be exactly 6 elems/partition => 1 block/call
                # v2 stats for the first act_k blocks of each supertile go to
                # the Scalar engine instead (raw sum + sum-of-squares).
                eff_act_k = 0 if no_act else act_k
                if eff_act_k > 0:
                    cpy = actpool.tile([P, act_k, D], fin, tag="cpy")
                    cpy2 = actpool.tile([P, act_k, D], fin, tag="cpy2")
                if not no_dve:
                    for h in range(0, SUPER, bn_batch):
                        g = s * SUPER + h
                        nc.vector.bn_stats(
                            out=s1[:, g : g + bn_batch, :],
                            in_=t1[:, h : h + bn_batch, :])
                for h in range(SUPER):
                    g = s * SUPER + h
                    if h < eff_act_k:
                        nc.scalar.activation(
                            out=cpy[:, h, :], in_=t2[:, h, :],
                            func=mybir.ActivationFunctionType.Copy,
                            accum_out=syB[:, g : g + 1])
                        nc.scalar.activation(
                            out=cpy2[:, h, :], in_=t2[:, h, :],
                            func=mybir.ActivationFunctionType.Square,
                            accum_out=syyB[:, g : g + 1])
                if not no_dve:
                    for h in range(eff_act_k, SUPER, bn_batch):
                        g = s * SUPER + h
                        hb = min(bn_batch, SUPER - h)
                        nc.vector.bn_stats(
                            out=s2[:, g : g + hb, :],
                            in_=t2[:, h : h + hb, :])

                # w = x + y on GPSIMD (keeps DVE free); sum(w^2) per row on ACT.
                # Sxy is recovered in the combine via the polarization identity.
                w = scratch.tile([P, SUPER, D], fin, tag="w")
                if not act_psum and not inplace_sq:
                    wsq = scratch.tile([P, SUPER, D], fin, tag="wsq")
                if not no_gp:
                    # first dve_k blocks of each supertile: add on DVE instead
                    # of GPSIMD (balances the two engines under the DMA time)
                    if dve_k > 0:
                        dsl = slice(0, dve_k)
                        nc.vector.tensor_tensor(
                            out=w[:, dsl, :], in0=t1[:, dsl, :],
                            in1=t2[:, dsl, :], op=mybir.AluOpType.add)
                    gsl = slice(dve_k, SUPER)
                    if dve_k >= SUPER:
                        pass
                    elif gp_split <= 1:
                        nc.gpsimd.tensor_tensor(
                            out=w[:, gsl, :], in0=t1[:, gsl, :],
                            in1=t2[:, gsl, :], op=mybir.AluOpType.add)
                    else:
                        hstep = (SUPER - dve_k) // gp_split
                        for j in range(gp_split):
                            js = slice(dve_k + j * hstep, dve_k + (j + 1) * hstep)
                            nc.gpsimd.tensor_tensor(
                                out=w[:, js, :], in0=t1[:, js, :],
                                in1=t2[:, js, :], op=mybir.AluOpType.add)
                if not no_act:
                    wsrc = t1 if no_gp else w
                    for b in range(SUPER):
                        i = s * SUPER + b
                        if act_psum:
                            wsqb = psum.tile([P, D], f32, tag="wsq")
                        elif inplace_sq:
                            wsqb = wsrc[:, b, :]
                        else:
                            wsqb = wsq[:, b, :]
                        nc.scalar.activation(
                            out=wsqb, in_=wsrc[:, b, :],
                            func=mybir.ActivationFunctionType.Square,
                            accum_out=sww[:, i : i + 1],
                        )

            # ---- final combine (all [128, NBLK] elementwise) ----
            # bn_stats 6-tuple: [n_e, mean_e, n_e*var_e, n_o, mean_o, n_o*var_o]
            # (even/odd element split, n_e = n_o = D/2)
            # mean  = (mean_e + mean_o)/2
            # M2    = n_e*var_e + n_o*var_o + (D/4)*(mean_e - mean_o)^2
            # num   = Sxy/D - mean1*mean2
            # r     = num * (D-1) / sqrt(M2x * M2y)
            cmb = stats
            m1 = cmb.tile([P, NBLK], f32)
            m2 = cmb.tile([P, NBLK], f32)
            m2x = cmb.tile([P, NBLK], f32)
            m2y = cmb.tile([P, NBLK], f32)
            tmp = cmb.tile([P, NBLK], f32)
            tmp2 = cmb.tile([P, NBLK], f32)
            res = cmb.tile([P, NBLK], f32)

            add = mybir.AluOpType.add
            sub = mybir.AluOpType.subtract
            mul = mybir.AluOpType.mult

            def v2view(x):
                # non-offloaded columns of a [P, NBLK] buffer (i % SUPER >= act_k)
                return x[:, :].rearrange("p (n h) -> p n h", h=SUPER)[:, :, act_k:]

            def v2view6(x):
                return x[:, :, :].rearrange("p (n h) c -> p n h c", h=SUPER)[:, :, act_k:, :]

            for (sbuf, mean, m2sum) in ((s1, m1, m2x), (s2, m2, m2y)):
                if act_k > 0 and sbuf is s2:
                    sbuf = v2view6(sbuf)
                    mean = v2view(mean)
                    m2sum = v2view(m2sum)
                    ttmp = v2view(tmp)
                    ttmp2 = v2view(tmp2)
                else:
                    ttmp = tmp
                    ttmp2 = tmp2
                fe_m = sbuf[:, :, 1] if len(sbuf.shape) == 3 else sbuf[:, :, :, 1]
                fo_m = sbuf[:, :, 4] if len(sbuf.shape) == 3 else sbuf[:, :, :, 4]
                fe_v = sbuf[:, :, 2] if len(sbuf.shape) == 3 else sbuf[:, :, :, 2]
                fo_v = sbuf[:, :, 5] if len(sbuf.shape) == 3 else sbuf[:, :, :, 5]
                # mean = 0.5*(fe_m + fo_m)
                nc.vector.tensor_tensor(out=ttmp, in0=fe_m, in1=fo_m, op=add)
                nc.vector.tensor_scalar_mul(out=mean, in0=ttmp, scalar1=0.5)
                # M2 = fe_v + fo_v + (D/4)*(fe_m - fo_m)^2
                nc.vector.tensor_tensor(out=ttmp, in0=fe_m, in1=fo_m, op=sub)
                nc.vector.tensor_tensor(out=ttmp, in0=ttmp, in1=ttmp, op=mul)
                nc.vector.tensor_tensor(out=ttmp2, in0=fe_v, in1=fo_v, op=add)
                nc.vector.scalar_tensor_tensor(
                    out=m2sum, in0=ttmp, scalar=float(D) / 4.0, in1=ttmp2,
                    op0=mul, op1=add,
                )

            if act_k > 0:
                # Offloaded columns (i % SUPER < act_k): m2 = Sy/D,
                # M2y = Syy - D*m2^2, written through 3D strided views.
                ksl = slice(0, act_k)
                m2v = m2[:, :].rearrange("p (n h) -> p n h", h=SUPER)[:, :, ksl]
                m2yv = m2y[:, :].rearrange("p (n h) -> p n h", h=SUPER)[:, :, ksl]
                syv = syB[:, :].rearrange("p (n h) -> p n h", h=SUPER)[:, :, ksl]
                syyv = syyB[:, :].rearrange("p (n h) -> p n h", h=SUPER)[:, :, ksl]
                tmpv = tmp[:, :].rearrange("p (n h) -> p n h", h=SUPER)[:, :, ksl]
                nc.vector.tensor_scalar_mul(out=m2v, in0=syv, scalar1=1.0 / float(D))
                nc.vector.tensor_tensor(out=tmpv, in0=m2v, in1=m2v, op=mul)
                nc.vector.scalar_tensor_tensor(
                    out=m2yv, in0=tmpv, scalar=-float(D), in1=syyv,
                    op0=mul, op1=add,
                )

            # Sxy = (Sww - Sxx - Syy)/2,  Sxx = M2x + D*m1^2, Syy = M2y + D*m2^2
            # num = Sxy/D - m1*m2
            #     = (Sww - M2x - M2y)/(2D) - (m1^2 + m2^2)/2 - m1*m2
            #     = (Sww - M2x - M2y)/(2D) - (m1 + m2)^2 / 2
            nc.vector.tensor_tensor(out=tmp, in0=sww, in1=m2x, op=sub)
            nc.vector.tensor_tensor(out=tmp, in0=tmp, in1=m2y, op=sub)
            nc.vector.tensor_tensor(out=tmp2, in0=m1, in1=m2, op=add)
            nc.vector.tensor_tensor(out=tmp2, in0=tmp2, in1=tmp2, op=mul)
            nc.vector.tensor_scalar_mul(out=tmp2, in0=tmp2, scalar1=0.5)
            nc.vector.scalar_tensor_tensor(
                out=tmp, in0=tmp, scalar=1.0 / (2.0 * float(D)), in1=tmp2,
                op0=mul, op1=sub,
            )
            # q = M2x*M2y ; r = num*(D-1)/sqrt(q)
            nc.vector.tensor_tensor(out=tmp2, in0=m2x, in1=m2y, op=mul)
            nc.scalar.sqrt(out=tmp2, in_=tmp2)
            nc.vector.reciprocal(out=tmp2, in_=tmp2)
            nc.vector.scalar_tensor_tensor(
                out=res, in0=tmp, scalar=float(D - 1), in1=tmp2,
                op0=mul, op1=mul,
            )
            nc.sync.dma_start(out=out[:], in_=res)

    nc.compile()
    return nc


def _build_v2(passes=1, dma_only=False, compute_only=False,
              data_bufs=8, scratch_bufs=6, layout=None,
              hw_loop=True, unroll_reps=1, in_dtype=None, cmb_gp=False):
    """Design v2: fp16 inputs, rowmajor DMA, and per-engine split tuned to
    the measured primitive costs:
      - DVE: one bn_stats per interleaved 2-block pair (even positions =
        block h, odd = block h+1) for BOTH v1 and v2 -> exact per-block
        (mean, M2) straight from the 6-tuple (no even/odd merge math)
      - GPSIMD: w = x + y (one batched op per supertile)
      - ACT: Square+accum per block -> sum(w^2); cross term recovered via
        the polarization identity in the combine
    Steady-state targets: DVE ~151us, ACT ~140us, GP ~119us, DMA ~90us.
    """
    import contextlib
    from concourse import bacc, mybir
    import concourse.tile as tile

    f32 = mybir.dt.float32
    in_dtype = in_dtype if in_dtype is not None else IN_DTYPE
    fin = getattr(mybir.dt, in_dtype)
    NPAIR = NBLK // 2
    nc = bacc.Bacc(None, target_bir_lowering=False, debug=False)

    v1 = nc.dram_tensor("v1", [N_PER_CORE, D], fin, kind="ExternalInput")
    v2 = nc.dram_tensor("v2", [N_PER_CORE, D], fin, kind="ExternalInput")
    out = nc.dram_tensor("out", [P, NBLK], f32, kind="ExternalOutput")

    layout = layout if layout is not None else LAYOUT
    if layout == "colmajor":
        v1r = v1[:].rearrange("(n p) d -> p n d", p=P)
        v2r = v2[:].rearrange("(n p) d -> p n d", p=P)
    else:
        v1r = v1[:].rearrange("(p n) d -> p n d", p=P)
        v2r = v2[:].rearrange("(p n) d -> p n d", p=P)

    with tile.TileContext(nc) as tc:
        with (
            tc.tile_pool(name="data", bufs=data_bufs) as data,
            tc.tile_pool(name="scratch", bufs=scratch_bufs) as scratch,
            tc.tile_pool(name="stats", bufs=1) as stats,
        ):
            s1 = stats.tile([P, NPAIR, 6], f32)  # pair-tuples of v1
            s2 = stats.tile([P, NPAIR, 6], f32)  # pair-tuples of v2
            sww = stats.tile([P, NBLK], f32)     # per-row sum((x+y)^2)
            if dma_only or compute_only:
                nc.vector.memset(s1, 1.0)
                nc.vector.memset(s2, 1.0)
                nc.vector.memset(sww, 1.0)

            if compute_only:
                t1c = data.tile([P, SUPER, D], fin, tag="t1")
                t2c = data.tile([P, SUPER, D], fin, tag="t2")
                nc.gpsimd.memset(t1c, 0.5)
                nc.gpsimd.memset(t2c, 0.25)

            if passes > 1 and hw_loop:
                rep_iter = range(unroll_reps)
                assert passes % unroll_reps == 0
                loop_cm = tc.For_i(0, passes // unroll_reps)
            else:
                rep_iter = range(passes)
                loop_cm = contextlib.nullcontext()
            with loop_cm:
             for _rep in rep_iter:
              for s in range(NSUP):
                blk = slice(s * SUPER, (s + 1) * SUPER)
                if compute_only:
                    t1, t2 = t1c, t2c
                else:
                    t1 = data.tile([P, SUPER, D], fin, tag="t1")
                    t2 = data.tile([P, SUPER, D], fin, tag="t2")
                    nc.sync.dma_start(out=t1, in_=v1r[:, blk, :])
                    nc.sync.dma_start(out=t2, in_=v2r[:, blk, :])
                if dma_only:
                    continue

                # DVE: one bn_stats per interleaved block-pair (raw
                # instruction; the wrapper rejects the [P, D, 2] view).
                for h in range(0, SUPER, 2):
                    gp_ = (s * SUPER + h) // 2
                    for tt, ss in ((t1, s1), (t2, s2)):
                        iv = tt[:, h : h + 2, :].rearrange("p h d -> p d h")
                        eng = nc.vector
                        eng.add_instruction(mybir.InstBNStats(
                            name=nc.get_next_instruction_name(),
                            ins=[eng.lower_ap(iv)],
                            outs=[eng.lower_ap(ss[:, gp_, : 6])],
                        ))

                # GPSIMD: w = x + y, one batched op
                w = scratch.tile([P, SUPER, D], fin, tag="w")
                wsq = scratch.tile([P, SUPER, D], fin, tag="wsq")
                nc.gpsimd.tensor_tensor(
                    out=w, in0=t1, in1=t2, op=mybir.AluOpType.add)

                # ACT: per-block Square with fused row-accumulate
                for b in range(SUPER):
                    i = s * SUPER + b
                    nc.scalar.activation(
                        out=wsq[:, b, :], in_=w[:, b, :],
                        func=mybir.ActivationFunctionType.Square,
                        accum_out=sww[:, i : i + 1],
                    )

            # ---- final combine ----
            # pair-tuple: [n_e, mean_e, M2_e, n_o, mean_o, M2_o] where the
            # even stats are block 2k and the odd stats block 2k+1.
            m1 = stats.tile([P, NBLK], f32)
            m2 = stats.tile([P, NBLK], f32)
            m2x = stats.tile([P, NBLK], f32)
            m2y = stats.tile([P, NBLK], f32)
            tmp = stats.tile([P, NBLK], f32)
            tmp2 = stats.tile([P, NBLK], f32)
            res = stats.tile([P, NBLK], f32)

            add = mybir.AluOpType.add
            sub = mybir.AluOpType.subtract
            mul = mybir.AluOpType.mult

            for (sp, mean_t, m2_t) in ((s1, m1, m2x), (s2, m2, m2y)):
                mv = mean_t[:, :].rearrange("p (n two) -> p n two", two=2)
                vv = m2_t[:, :].rearrange("p (n two) -> p n two", two=2)
                nc.scalar.copy(out=mv[:, :, 0], in_=sp[:, :, 1])
                nc.scalar.copy(out=mv[:, :, 1], in_=sp[:, :, 4])
                nc.scalar.copy(out=vv[:, :, 0], in_=sp[:, :, 2])
                nc.scalar.copy(out=vv[:, :, 1], in_=sp[:, :, 5])

            # num = (Sww - M2x - M2y)/(2D) - (m1 + m2)^2 / 2
            eng_tt = nc.gpsimd if cmb_gp else nc.vector
            eng_tt.tensor_tensor(out=tmp, in0=sww, in1=m2x, op=sub)
            eng_tt.tensor_tensor(out=tmp, in0=tmp, in1=m2y, op=sub)
            eng_tt.tensor_tensor(out=tmp2, in0=m1, in1=m2, op=add)
            nc.vector.tensor_tensor(out=tmp2, in0=tmp2, in1=tmp2, op=mul)
            nc.vector.tensor_scalar_mul(out=tmp2, in0=tmp2, scalar1=0.5)
            nc.vector.scalar_tensor_tensor(
                out=tmp, in0=tmp, scalar=1.0 / (2.0 * float(D)), in1=tmp2,
                op0=mul, op1=sub,
            )
            # r = num * (D-1) / sqrt(M2x * M2y)
            nc.vector.tensor_tensor(out=tmp2, in0=m2x, in1=m2y, op=mul)
            nc.scalar.sqrt(out=tmp2, in_=tmp2)
            nc.vector.reciprocal(out=tmp2, in_=tmp2)
            nc.vector.scalar_tensor_tensor(
                out=res, in0=tmp, scalar=float(D - 1), in1=tmp2,
                op0=mul, op1=mul,
            )
            nc.sync.dma_start(out=out[:], in_=res)

    nc.compile()
    return nc


DESIGN = "v2"


def _get_nc():
    global _NC_CACHE
    if _NC_CACHE is None:
        _NC_CACHE = _build_v2() if DESIGN == "v2" else _build_nc()
    return _NC_CACHE


def _run(v1, v2, trace=False):
    from concourse.bass_utils import run_bass_kernel_spmd

    nc = _get_nc()
    v1 = to_input_dtype(np.asarray(v1))
    v2 = to_input_dtype(np.asarray(v2))
    assert v1.shape == (N_FULL, D) and v2.shape == (N_FULL, D)

    in_maps = []
    for c in range(N_CORES):
        sl = slice(c * N_PER_CORE, (c + 1) * N_PER_CORE)
        in_maps.append({
            "v1": np.ascontiguousarray(v1[sl]),
            "v2": np.ascontiguousarray(v2[sl]),
        })
    res = run_bass_kernel_spmd(
        nc, in_maps, core_ids=list(range(N_CORES)), trace=trace
    )
    if LAYOUT == "colmajor":
        # out[p, i] -> row i*128 + p  =>  per-core flat = out.T.reshape(-1)
        parts = [np.asarray(r["out"]).T.reshape(-1) for r in res.results]
    else:
        # out[p, i] -> row p*NBLK + i  =>  per-core flat = out.reshape(-1)
        parts = [np.asarray(r["out"]).reshape(-1) for r in res.results]
    full = np.concatenate(parts)
    return full, res


def kernel(v1, v2):
    out, _ = _run(v1, v2, trace=False)
    return out



# revision 37
# speedup vs baseline: 1.0114x; 1.0114x over previous
"""Row-wise Pearson correlation kernel for Trainium2 (Bass/Tile).

Full inputs v1, v2: [262144, 256] f32. Output: [262144] f32 where
out[r] = (E[xy] - E[x]E[y]) / sqrt(var_s(x) * var_s(y))   (sample var, ddof=1)

Sharding: rows split evenly across 8 NeuronCores (no communication needed).
Per core: 32768 rows -> 256 blocks of 128 rows (rows on partitions).

Design (v2, ~145us/pass steady state vs ~196us for the f32 baseline):
  - HOST casts f32 -> fp16 before upload (rel tol is 2e-2; fp16 adds only
    ~7e-4 rel error). Halves HBM traffic: DMA floor ~90us/core.
  - rowmajor DRAM layout: row r = p*NBLK + i; per-partition reads are
    SUPER*512B contiguous runs -> ~374 GB/s/core (~3.0 TB/s aggregate).
  - VectorE: ONE bn_stats per interleaved 2-block pair ([P, D, 2] view;
    even stream positions = block 2k, odd = block 2k+1). The bn 6-tuple's
    native even/odd split then yields EXACT per-block (mean, M2) -- two
    blocks per instruction, no even/odd merge math. ~74us per tensor.
  - GPSIMD: w = v1 + v2, one batched op per supertile (~119us).
  - ScalarE: per-block Square-activation with fused row-accumulate ->
    sum(w^2) (~140us); Sxy recovered in the combine via polarization:
      num = (Sww - M2x - M2y)/(2D) - (mx + my)^2/2
  - tail combine on [128, 256] stat tiles -> r; chunked DMA out.
Engine balance: DVE ~148, ACT ~140, GP ~119, DMA ~90 -> compute-bound at
~145us; measured via For_i hw-loop slope (passes 8 vs 512, min-based).
"""

import numpy as np

N_FULL = 262144
D = 256
N_CORES = 8
N_PER_CORE = N_FULL // N_CORES  # 32768
P = 128
NBLK = N_PER_CORE // P          # 256 row-blocks per core
SUPER = 8                       # row-blocks per supertile (one DMA / bn_stats batch)
NSUP = NBLK // SUPER

_NC_CACHE = None
LAYOUT = "rowmajor"  # or "colmajor"
IN_DTYPE = "float16"  # HBM storage dtype for v1/v2 (host casts f32 inputs)


def to_input_dtype(arr):
    import numpy as np
    if IN_DTYPE == "float16":
        return np.ascontiguousarray(arr, dtype=np.float16)
    if IN_DTYPE == "bfloat16":
        import ml_dtypes
        return np.ascontiguousarray(np.asarray(arr).astype(ml_dtypes.bfloat16))
    return np.ascontiguousarray(arr, dtype=np.float32)


def _build_nc(passes=1, dma_only=False, compute_only=False,
              super_=None, data_bufs=6, act_k=1, scratch_bufs=4, dma_split=1,
              layout=None, act_psum=False, inplace_sq=False, gp_split=1,
              hw_loop=True, unroll_reps=1, dve_k=0, in_dtype=None,
              no_dve=False, no_act=False, no_gp=False, bn_batch=1,
              probe=None):
    """passes>1 with hw_loop=True wraps the streaming loop in a tc.For_i
    hardware loop (constant program size, used for slope timing)."""
    import contextlib
    from concourse import bacc, mybir
    import concourse.tile as tile

    f32 = mybir.dt.float32
    in_dtype = in_dtype if in_dtype is not None else IN_DTYPE
    fin = getattr(mybir.dt, in_dtype)
    SUPER = super_ if super_ is not None else globals()["SUPER"]
    NSUP = NBLK // SUPER
    nc = bacc.Bacc(None, target_bir_lowering=False, debug=False)

    v1 = nc.dram_tensor("v1", [N_PER_CORE, D], fin, kind="ExternalInput")
    v2 = nc.dram_tensor("v2", [N_PER_CORE, D], fin, kind="ExternalInput")
    # out[p, i] = r(row i*128 + p); host transposes back
    out = nc.dram_tensor("out", [P, NBLK], f32, kind="ExternalOutput")

    layout = layout if layout is not None else LAYOUT
    if layout == "colmajor":
        # out[p, i] = r(row i*128 + p); per-partition DRAM reads are 1KB chunks
        v1r = v1[:].rearrange("(n p) d -> p n d", p=P)  # [128, NBLK, D]
        v2r = v2[:].rearrange("(n p) d -> p n d", p=P)
    else:
        # rowmajor: out[p, i] = r(row p*NBLK + i); per-partition DRAM reads are
        # SUPER*1KB contiguous chunks (fewer, larger descriptor runs)
        v1r = v1[:].rearrange("(p n) d -> p n d", p=P)  # [128, NBLK, D]
        v2r = v2[:].rearrange("(p n) d -> p n d", p=P)

    with tile.TileContext(nc) as tc:
        with (
            tc.tile_pool(name="data", bufs=data_bufs) as data,
            tc.tile_pool(name="scratch", bufs=scratch_bufs) as scratch,
            tc.tile_pool(name="stats", bufs=1) as stats,
            tc.tile_pool(name="psum", bufs=2, space="PSUM") as psum,
        ):
            actpool = psum if act_psum else scratch
            s1 = stats.tile([P, NBLK, 6], f32)   # bn_stats(v1): per-block 6-tuple
            s2 = stats.tile([P, NBLK, 6], f32)
            sww = stats.tile([P, NBLK], f32)     # per-row sum((x+y)^2)
            syB = stats.tile([P, NBLK], f32)     # ACT-offloaded: raw sum(y)
            syyB = stats.tile([P, NBLK], f32)    # ACT-offloaded: raw sum(y^2)
            if dma_only or compute_only or no_dve or no_act:
                nc.vector.memset(s1, 1.0)
                nc.vector.memset(s2, 1.0)
                nc.vector.memset(sww, 1.0)
            if dma_only or compute_only or act_k > 0 or no_act:
                nc.vector.memset(syB, 1.0)
                nc.vector.memset(syyB, 1.0)

            if compute_only:
                t1c = data.tile([P, SUPER, D], fin, tag="t1")
                t2c = data.tile([P, SUPER, D], fin, tag="t2")
                nc.gpsimd.memset(t1c, 0.5)
                nc.gpsimd.memset(t2c, 0.25)

            if passes > 1 and hw_loop:
                rep_iter = range(unroll_reps)
                assert passes % unroll_reps == 0
                loop_cm = tc.For_i(0, passes // unroll_reps)
            else:
                rep_iter = range(passes)
                loop_cm = contextlib.nullcontext()
            with loop_cm:
             for _rep in rep_iter:
              for s in range(NSUP):
                blk = slice(s * SUPER, (s + 1) * SUPER)
                if compute_only:
                    t1, t2 = t1c, t2c
                else:
                    t1 = data.tile([P, SUPER, D], fin, tag="t1")
                    t2 = data.tile([P, SUPER, D], fin, tag="t2")
                    if dma_split <= 1:
                        nc.sync.dma_start(out=t1, in_=v1r[:, blk, :])
                        nc.sync.dma_start(out=t2, in_=v2r[:, blk, :])
                    else:
                        step = SUPER // dma_split
                        for j in range(dma_split):
                            jb = slice(s * SUPER + j * step, s * SUPER + (j + 1) * step)
                            jt = slice(j * step, (j + 1) * step)
                            nc.sync.dma_start(out=t1[:, jt, :], in_=v1r[:, jb, :])
                            nc.sync.dma_start(out=t2[:, jt, :], in_=v2r[:, jb, :])
                if dma_only:
                    continue

                if probe is not None:
                    # engine-rate microbenchmarks, one primitive per probe
                    if probe == "dve_reduce":
                        redout = scratch.tile([P, 3, SUPER], f32, tag="red")
                        for k, tt in enumerate((t1, t2, t1)):
                            nc.vector.tensor_reduce(
                                out=redout[:, k, :], in_=tt,
                                axis=mybir.AxisListType.X,
                                op=mybir.AluOpType.add)
                    elif probe == "dve_tt":
                        wp = scratch.tile([P, SUPER, D], fin, tag="w")
                        nc.vector.tensor_tensor(
                            out=wp, in0=t1, in1=t2, op=mybir.AluOpType.add)
                    elif probe == "act_batch":
                        wp = scratch.tile([P, SUPER, D], fin, tag="w")
                        wp2 = scratch.tile([P, SUPER, D], fin, tag="wsq")
                        nc.scalar.activation(
                            out=wp, in_=t1,
                            func=mybir.ActivationFunctionType.Square)
                        nc.scalar.activation(
                            out=wp2, in_=t2,
                            func=mybir.ActivationFunctionType.Square)
                    elif probe == "gp_mult":
                        wp = scratch.tile([P, SUPER, D], fin, tag="w")
                        nc.gpsimd.tensor_tensor(
                            out=wp, in0=t1, in1=t2, op=mybir.AluOpType.mult)
                    elif probe == "gp_stt":
                        wp = scratch.tile([P, SUPER, D], fin, tag="w")
                        for h in range(SUPER):
                            g = s * SUPER + h
                            nc.gpsimd.scalar_tensor_tensor(
                                out=wp[:, h, :], in0=t1[:, h, :], scalar=1.0,
                                in1=t2[:, h, :],
                                op0=mybir.AluOpType.mult,
                                op1=mybir.AluOpType.mult,
                                accum_out=sww[:, g : g + 1])
                    elif probe == "dve_bn2":
                        # one bn_stats over an interleaved 2-block view:
                        # even positions = block h, odd = block h+1.
                        # Bypasses the wrapper's segmented-shape check (the HW
                        # emits ONE 6-tuple per instruction over the stream).
                        for h in range(0, SUPER, 2):
                            g = s * SUPER + h
                            for tt, ss in ((t1, s1), (t2, s2)):
                                iv = tt[:, h : h + 2, :].rearrange("p h d -> p d h")
                                eng = nc.vector
                                eng.add_instruction(mybir.InstBNStats(
                                    name=nc.get_next_instruction_name(),
                                    ins=[eng.lower_ap(iv)],
                                    outs=[eng.lower_ap(ss[:, g, : 6])],
                                ))
                    elif probe == "dve_ttred":
                        wp = scratch.tile([P, SUPER, D], fin, tag="w")
                        for h in range(SUPER):
                            g = s * SUPER + h
                            nc.vector.tensor_tensor_reduce(
                                out=wp[:, h, :], in0=t1[:, h, :],
                                in1=t2[:, h, :], scale=1.0, scalar=0.0,
                                op0=mybir.AluOpType.mult,
                                op1=mybir.AluOpType.add,
                                accum_out=sww[:, g : g + 1])
                    else:
                        raise ValueError(probe)
                    continue

                # bn_stats output must be exactly 6 elems/partition => 1 block/call
                # v2 stats for the first act_k blocks of each supertile go to
                # the Scalar engine instead (raw sum + sum-of-squares).
                eff_act_k = 0 if no_act else act_k
                if eff_act_k > 0:
                    cpy = actpool.tile([P, act_k, D], fin, tag="cpy")
                    cpy2 = actpool.tile([P, act_k, D], fin, tag="cpy2")
                if not no_dve:
                    for h in range(0, SUPER, bn_batch):
                        g = s * SUPER + h
                        nc.vector.bn_stats(
                            out=s1[:, g : g + bn_batch, :],
                            in_=t1[:, h : h + bn_batch, :])
                for h in range(SUPER):
                    g = s * SUPER + h
                    if h < eff_act_k:
                        nc.scalar.activation(
                            out=cpy[:, h, :], in_=t2[:, h, :],
                            func=mybir.ActivationFunctionType.Copy,
                            accum_out=syB[:, g : g + 1])
                        nc.scalar.activation(
                            out=cpy2[:, h, :], in_=t2[:, h, :],
                            func=mybir.ActivationFunctionType.Square,
                            accum_out=syyB[:, g : g + 1])
                if not no_dve:
                    for h in range(eff_act_k, SUPER, bn_batch):
                        g = s * SUPER + h
                        hb = min(bn_batch, SUPER - h)
                        nc.vector.bn_stats(
                            out=s2[:, g : g + hb, :],
                            in_=t2[:, h : h + hb, :])

                # w = x + y on GPSIMD (keeps DVE free); sum(w^2) per row on ACT.
                # Sxy is recovered in the combine via the polarization identity.
                w = scratch.tile([P, SUPER, D], fin, tag="w")
                if not act_psum and not inplace_sq:
                    wsq = scratch.tile([P, SUPER, D], fin, tag="wsq")
                if not no_gp:
                    # first dve_k blocks of each supertile: add on DVE instead
                    # of GPSIMD (balances the two engines under the DMA time)
                    if dve_k > 0:
                        dsl = slice(0, dve_k)
                        nc.vector.tensor_tensor(
                            out=w[:, dsl, :], in0=t1[:, dsl, :],
                            in1=t2[:, dsl, :], op=mybir.AluOpType.add)
                    gsl = slice(dve_k, SUPER)
                    if dve_k >= SUPER:
                        pass
                    elif gp_split <= 1:
                        nc.gpsimd.tensor_tensor(
                            out=w[:, gsl, :], in0=t1[:, gsl, :],
                            in1=t2[:, gsl, :], op=mybir.AluOpType.add)
                    else:
                        hstep = (SUPER - dve_k) // gp_split
                        for j in range(gp_split):
                            js = slice(dve_k + j * hstep, dve_k + (j + 1) * hstep)
                            nc.gpsimd.tensor_tensor(
                                out=w[:, js, :], in0=t1[:, js, :],
                                in1=t2[:, js, :], op=mybir.AluOpType.add)
                if not no_act:
                    wsrc = t1 if no_gp else w
                    for b in range(SUPER):
                        i = s * SUPER + b
                        if act_psum:
                            wsqb = psum.tile([P, D], f32, tag="wsq")
                        elif inplace_sq:
                            wsqb = wsrc[:, b, :]
                        else:
                            wsqb = wsq[:, b, :]
                        nc.scalar.activation(
                            out=wsqb, in_=wsrc[:, b, :],
                            func=mybir.ActivationFunctionType.Square,
                            accum_out=sww[:, i : i + 1],
                        )

            # ---- final combine (all [128, NBLK] elementwise) ----
            # bn_stats 6-tuple: [n_e, mean_e, n_e*var_e, n_o, mean_o, n_o*var_o]
            # (even/odd element split, n_e = n_o = D/2)
            # mean  = (mean_e + mean_o)/2
            # M2    = n_e*var_e + n_o*var_o + (D/4)*(mean_e - mean_o)^2
            # num   = Sxy/D - mean1*mean2
            # r     = num * (D-1) / sqrt(M2x * M2y)
            cmb = stats
            m1 = cmb.tile([P, NBLK], f32)
            m2 = cmb.tile([P, NBLK], f32)
            m2x = cmb.tile([P, NBLK], f32)
            m2y = cmb.tile([P, NBLK], f32)
            tmp = cmb.tile([P, NBLK], f32)
            tmp2 = cmb.tile([P, NBLK], f32)
            res = cmb.tile([P, NBLK], f32)

            add = mybir.AluOpType.add
            sub = mybir.AluOpType.subtract
            mul = mybir.AluOpType.mult

            def v2view(x):
                # non-offloaded columns of a [P, NBLK] buffer (i % SUPER >= act_k)
                return x[:, :].rearrange("p (n h) -> p n h", h=SUPER)[:, :, act_k:]

            def v2view6(x):
                return x[:, :, :].rearrange("p (n h) c -> p n h c", h=SUPER)[:, :, act_k:, :]

            for (sbuf, mean, m2sum) in ((s1, m1, m2x), (s2, m2, m2y)):
                if act_k > 0 and sbuf is s2:
                    sbuf = v2view6(sbuf)
                    mean = v2view(mean)
                    m2sum = v2view(m2sum)
                    ttmp = v2view(tmp)
                    ttmp2 = v2view(tmp2)
                else:
                    ttmp = tmp
                    ttmp2 = tmp2
                fe_m = sbuf[:, :, 1] if len(sbuf.shape) == 3 else sbuf[:, :, :, 1]
                fo_m = sbuf[:, :, 4] if len(sbuf.shape) == 3 else sbuf[:, :, :, 4]
                fe_v = sbuf[:, :, 2] if len(sbuf.shape) == 3 else sbuf[:, :, :, 2]
                fo_v = sbuf[:, :, 5] if len(sbuf.shape) == 3 else sbuf[:, :, :, 5]
                # mean = 0.5*(fe_m + fo_m)
                nc.vector.tensor_tensor(out=ttmp, in0=fe_m, in1=fo_m, op=add)
                nc.vector.tensor_scalar_mul(out=mean, in0=ttmp, scalar1=0.5)
                # M2 = fe_v + fo_v + (D/4)*(fe_m - fo_m)^2
                nc.vector.tensor_tensor(out=ttmp, in0=fe_m, in1=fo_m, op=sub)
                nc.vector.tensor_tensor(out=ttmp, in0=ttmp, in1=ttmp, op=mul)
                nc.vector.tensor_tensor(out=ttmp2, in0=fe_v, in1=fo_v, op=add)
                nc.vector.scalar_tensor_tensor(
                    out=m2sum, in0=ttmp, scalar=float(D) / 4.0, in1=ttmp2,
                    op0=mul, op1=add,
                )

            if act_k > 0:
                # Offloaded columns (i % SUPER < act_k): m2 = Sy/D,
                # M2y = Syy - D*m2^2, written through 3D strided views.
                ksl = slice(0, act_k)
                m2v = m2[:, :].rearrange("p (n h) -> p n h", h=SUPER)[:, :, ksl]
                m2yv = m2y[:, :].rearrange("p (n h) -> p n h", h=SUPER)[:, :, ksl]
                syv = syB[:, :].rearrange("p (n h) -> p n h", h=SUPER)[:, :, ksl]
                syyv = syyB[:, :].rearrange("p (n h) -> p n h", h=SUPER)[:, :, ksl]
                tmpv = tmp[:, :].rearrange("p (n h) -> p n h", h=SUPER)[:, :, ksl]
                nc.vector.tensor_scalar_mul(out=m2v, in0=syv, scalar1=1.0 / float(D))
                nc.vector.tensor_tensor(out=tmpv, in0=m2v, in1=m2v, op=mul)
                nc.vector.scalar_tensor_tensor(
                    out=m2yv, in0=tmpv, scalar=-float(D), in1=syyv,
                    op0=mul, op1=add,
                )

            # Sxy = (Sww - Sxx - Syy)/2,  Sxx = M2x + D*m1^2, Syy = M2y + D*m2^2
            # num = Sxy/D - m1*m2
            #     = (Sww - M2x - M2y)/(2D) - (m1^2 + m2^2)/2 - m1*m2
            #     = (Sww - M2x - M2y)/(2D) - (m1 + m2)^2 / 2
            nc.vector.tensor_tensor(out=tmp, in0=sww, in1=m2x, op=sub)
            nc.vector.tensor_tensor(out=tmp, in0=tmp, in1=m2y, op=sub)
            nc.vector.tensor_tensor(out=tmp2, in0=m1, in1=m2, op=add)
            nc.vector.tensor_tensor(out=tmp2, in0=tmp2, in1=tmp2, op=mul)
            nc.vector.tensor_scalar_mul(out=tmp2, in0=tmp2, scalar1=0.5)
            nc.vector.scalar_tensor_tensor(
                out=tmp, in0=tmp, scalar=1.0 / (2.0 * float(D)), in1=tmp2,
                op0=mul, op1=sub,
            )
            # q = M2x*M2y ; r = num*(D-1)/sqrt(q)
            nc.vector.tensor_tensor(out=tmp2, in0=m2x, in1=m2y, op=mul)
            nc.scalar.sqrt(out=tmp2, in_=tmp2)
            nc.vector.reciprocal(out=tmp2, in_=tmp2)
            nc.vector.scalar_tensor_tensor(
                out=res, in0=tmp, scalar=float(D - 1), in1=tmp2,
                op0=mul, op1=mul,
            )
            nc.sync.dma_start(out=out[:], in_=res)

    nc.compile()
    return nc


def _build_v2(passes=1, dma_only=False, compute_only=False,
              data_bufs=8, scratch_bufs=6, layout=None,
              hw_loop=True, unroll_reps=1, in_dtype=None, cmb_gp=False,
              super_=None, sq_inplace=False):
    """Design v2: fp16 inputs, rowmajor DMA, and per-engine split tuned to
    the measured primitive costs:
      - DVE: one bn_stats per interleaved 2-block pair (even positions =
        block h, odd = block h+1) for BOTH v1 and v2 -> exact per-block
        (mean, M2) straight from the 6-tuple (no even/odd merge math)
      - GPSIMD: w = x + y (one batched op per supertile)
      - ACT: Square+accum per block -> sum(w^2); cross term recovered via
        the polarization identity in the combine
    Steady-state targets: DVE ~151us, ACT ~140us, GP ~119us, DMA ~90us.
    """
    import contextlib
    from concourse import bacc, mybir
    import concourse.tile as tile

    f32 = mybir.dt.float32
    in_dtype = in_dtype if in_dtype is not None else IN_DTYPE
    fin = getattr(mybir.dt, in_dtype)
    SUPER = super_ if super_ is not None else globals()["SUPER"]
    NSUP = NBLK // SUPER
    NPAIR = NBLK // 2
    nc = bacc.Bacc(None, target_bir_lowering=False, debug=False)

    v1 = nc.dram_tensor("v1", [N_PER_CORE, D], fin, kind="ExternalInput")
    v2 = nc.dram_tensor("v2", [N_PER_CORE, D], fin, kind="ExternalInput")
    out = nc.dram_tensor("out", [P, NBLK], f32, kind="ExternalOutput")

    layout = layout if layout is not None else LAYOUT
    if layout == "colmajor":
        v1r = v1[:].rearrange("(n p) d -> p n d", p=P)
        v2r = v2[:].rearrange("(n p) d -> p n d", p=P)
    else:
        v1r = v1[:].rearrange("(p n) d -> p n d", p=P)
        v2r = v2[:].rearrange("(p n) d -> p n d", p=P)

    with tile.TileContext(nc) as tc:
        with (
            tc.tile_pool(name="data", bufs=data_bufs) as data,
            tc.tile_pool(name="scratch", bufs=scratch_bufs) as scratch,
            tc.tile_pool(name="stats", bufs=1) as stats,
        ):
            s1 = stats.tile([P, NPAIR, 6], f32)  # pair-tuples of v1
            s2 = stats.tile([P, NPAIR, 6], f32)  # pair-tuples of v2
            sww = stats.tile([P, NBLK], f32)     # per-row sum((x+y)^2)
            if dma_only or compute_only:
                nc.vector.memset(s1, 1.0)
                nc.vector.memset(s2, 1.0)
                nc.vector.memset(sww, 1.0)

            if compute_only:
                t1c = data.tile([P, SUPER, D], fin, tag="t1")
                t2c = data.tile([P, SUPER, D], fin, tag="t2")
                nc.gpsimd.memset(t1c, 0.5)
                nc.gpsimd.memset(t2c, 0.25)

            # combine scratch ([P, NBLK] f32); written in per-group slices
            m1 = stats.tile([P, NBLK], f32)
            m2 = stats.tile([P, NBLK], f32)
            m2x = stats.tile([P, NBLK], f32)
            m2y = stats.tile([P, NBLK], f32)
            tmp = stats.tile([P, NBLK], f32)
            tmp2 = stats.tile([P, NBLK], f32)
            res = stats.tile([P, NBLK], f32)

            add = mybir.AluOpType.add
            sub = mybir.AluOpType.subtract
            mul = mybir.AluOpType.mult

            def combine_group(c0, c1):
                """Emit the combine for block columns [c0, c1). The tile
                dependency tracker overlaps this with later supertiles'
                stream work (it only depends on s1/s2/sww slices of this
                group).

                pair-tuple: [n_e, mean_e, M2_e, n_o, mean_o, M2_o]; even
                stats are block 2k, odd stats block 2k+1.
                """
                pc = slice(c0 // 2, c1 // 2)
                bc = slice(c0, c1)
                for (sp, mean_t, m2_t) in ((s1, m1, m2x), (s2, m2, m2y)):
                    mv = mean_t[:, bc].rearrange("p (n two) -> p n two", two=2)
                    vv = m2_t[:, bc].rearrange("p (n two) -> p n two", two=2)
                    nc.scalar.copy(out=mv[:, :, 0], in_=sp[:, pc, 1])
                    nc.scalar.copy(out=mv[:, :, 1], in_=sp[:, pc, 4])
                    nc.scalar.copy(out=vv[:, :, 0], in_=sp[:, pc, 2])
                    nc.scalar.copy(out=vv[:, :, 1], in_=sp[:, pc, 5])

                # num = (Sww - M2x - M2y)/(2D) - (m1 + m2)^2 / 2
                eng_tt = nc.gpsimd if cmb_gp else nc.vector
                eng_tt.tensor_tensor(out=tmp[:, bc], in0=sww[:, bc],
                                     in1=m2x[:, bc], op=sub)
                eng_tt.tensor_tensor(out=tmp[:, bc], in0=tmp[:, bc],
                                     in1=m2y[:, bc], op=sub)
                eng_tt.tensor_tensor(out=tmp2[:, bc], in0=m1[:, bc],
                                     in1=m2[:, bc], op=add)
                nc.vector.tensor_tensor(out=tmp2[:, bc], in0=tmp2[:, bc],
                                        in1=tmp2[:, bc], op=mul)
                nc.vector.tensor_scalar_mul(out=tmp2[:, bc], in0=tmp2[:, bc],
                                            scalar1=0.5)
                nc.vector.scalar_tensor_tensor(
                    out=tmp[:, bc], in0=tmp[:, bc],
                    scalar=1.0 / (2.0 * float(D)), in1=tmp2[:, bc],
                    op0=mul, op1=sub,
                )
                # r = num * (D-1) / sqrt(M2x * M2y)
                nc.vector.tensor_tensor(out=tmp2[:, bc], in0=m2x[:, bc],
                                        in1=m2y[:, bc], op=mul)
                nc.scalar.sqrt(out=tmp2[:, bc], in_=tmp2[:, bc])
                nc.vector.reciprocal(out=tmp2[:, bc], in_=tmp2[:, bc])
                nc.vector.scalar_tensor_tensor(
                    out=res[:, bc], in0=tmp[:, bc], scalar=float(D - 1),
                    in1=tmp2[:, bc], op0=mul, op1=mul,
                )
                nc.sync.dma_start(out=out[:, bc], in_=res[:, bc])

            GSUP = 8  # supertiles per combine group
            if passes > 1 and hw_loop:
                rep_iter = range(unroll_reps)
                assert passes % unroll_reps == 0
                loop_cm = tc.For_i(0, passes // unroll_reps)
            else:
                rep_iter = range(passes)
                loop_cm = contextlib.nullcontext()
            with loop_cm:
             for _rep in rep_iter:
              for s in range(NSUP):
                blk = slice(s * SUPER, (s + 1) * SUPER)
                if compute_only:
                    t1, t2 = t1c, t2c
                else:
                    t1 = data.tile([P, SUPER, D], fin, tag="t1")
                    t2 = data.tile([P, SUPER, D], fin, tag="t2")
                    nc.sync.dma_start(out=t1, in_=v1r[:, blk, :])
                    nc.sync.dma_start(out=t2, in_=v2r[:, blk, :])
                if not dma_only:
                    # DVE: one bn_stats per interleaved block-pair (raw
                    # instruction; the wrapper rejects the [P, D, 2] view).
                    for h in range(0, SUPER, 2):
                        gp_ = (s * SUPER + h) // 2
                        for tt, ss in ((t1, s1), (t2, s2)):
                            iv = tt[:, h : h + 2, :].rearrange("p h d -> p d h")
                            eng = nc.vector
                            eng.add_instruction(mybir.InstBNStats(
                                name=nc.get_next_instruction_name(),
                                ins=[eng.lower_ap(iv)],
                                outs=[eng.lower_ap(ss[:, gp_, : 6])],
                            ))

                    # GPSIMD: w = x + y, one batched op
                    w = scratch.tile([P, SUPER, D], fin, tag="w")
                    if not sq_inplace:
                        wsq = scratch.tile([P, SUPER, D], fin, tag="wsq")
                    nc.gpsimd.tensor_tensor(
                        out=w, in0=t1, in1=t2, op=mybir.AluOpType.add)

                    # ACT: per-block Square with fused row-accumulate
                    for b in range(SUPER):
                        i = s * SUPER + b
                        dst = w[:, b, :] if sq_inplace else wsq[:, b, :]
                        nc.scalar.activation(
                            out=dst, in_=w[:, b, :],
                            func=mybir.ActivationFunctionType.Square,
                            accum_out=sww[:, i : i + 1],
                        )

            # single tail combine: per-group interleaving was measured
            # SLOWER (whole-tile WAR tracking on sww serializes the stream)
            combine_group(0, NBLK)

    nc.compile()
    return nc


DESIGN = "v2"


def _get_nc():
    global _NC_CACHE
    if _NC_CACHE is None:
        _NC_CACHE = _build_v2() if DESIGN == "v2" else _build_nc()
    return _NC_CACHE


def _run(v1, v2, trace=False):
    from concourse.bass_utils import run_bass_kernel_spmd

    nc = _get_nc()
    v1 = to_input_dtype(np.asarray(v1))
    v2 = to_input_dtype(np.asarray(v2))
    assert v1.shape == (N_FULL, D) and v2.shape == (N_FULL, D)

    in_maps = []
    for c in range(N_CORES):
        sl = slice(c * N_PER_CORE, (c + 1) * N_PER_CORE)
        in_maps.append({
            "v1": np.ascontiguousarray(v1[sl]),
            "v2": np.ascontiguousarray(v2[sl]),
        })
    res = run_bass_kernel_spmd(
        nc, in_maps, core_ids=list(range(N_CORES)), trace=trace
    )
    if LAYOUT == "colmajor":
        # out[p, i] -> row i*128 + p  =>  per-core flat = out.T.reshape(-1)
        parts = [np.asarray(r["out"]).T.reshape(-1) for r in res.results]
    else:
        # out[p, i] -> row p*NBLK + i  =>  per-core flat = out.reshape(-1)
        parts = [np.asarray(r["out"]).reshape(-1) for r in res.results]
    full = np.concatenate(parts)
    return full, res


def kernel(v1, v2):
    out, _ = _run(v1, v2, trace=False)
    return out

